# revision 7
# baseline (speedup 1.0000x reference)
"""Converged-inhibition kernel for Trainium2 (8 NeuronCores, data-parallel).

The reference computes, per pixel (n, h, w):
    y = IFFT(FFT(x_c) / FFT(delta - filter)).real      (C = 63 channels)

Dividing by a fixed filter's DFT and inverse-transforming is a circular
deconvolution along the channel axis: y = G @ x with G the 63x63 circulant
matrix built from g = IFFT(1 / FFT(delta - filter)).real.  So the whole op
is one (63, 63) @ (63, N*H*W) matmul, embarrassingly parallel over pixels.

Device mapping: batch dim (64) sharded over 8 cores.  Since the contraction
dim (63) uses less than half the 128-wide PE array, two batches are stacked
per matmul column via a 126x126 block-diagonal weight, doubling PE
throughput.  The kernel is HBM-bandwidth bound (~50.6 MB per core).
"""

import os
import numpy as np

# Problem geometry (hardcoded: kernel.py must be self-contained).
C = 63
N_BATCH = 64
H = W = 112
HW = H * W                      # 12544
N_CORES = 8
B_PER_CORE = N_BATCH // N_CORES  # 8
P = 2 * C                       # 126 partitions = 2 batches stacked
ROWS = B_PER_CORE * C           # 504
N_GROUPS = B_PER_CORE // 2      # 4 batch-pairs per core
CHUNK = HW // 2                 # 6272 free-dim elements per pipelined chunk
N_CHUNKS = HW // CHUNK          # 2
MM_N = 512                      # moving free-dim per matmul (one PSUM bank)

# Matmul operand dtype: "float32" (safe) or "float32r" (fast fp32 PE path).
MM_DTYPE = os.environ.get("CI_MM_DTYPE", "float32")

_PROG_CACHE = {}


def _build_program(mm_dtype_name):
    import concourse.bacc as bacc
    import concourse.mybir as mybir
    from concourse import tile

    # Bacc (not raw Bass): its compile() splits multi-semaphore waits into
    # event-semaphore chains (HW allows only one wait per instruction).
    nc = bacc.Bacc("TRN2", target_bir_lowering=False, debug=False)
    x_d = nc.dram_tensor("x", [ROWS, HW], mybir.dt.float32, kind="ExternalInput").ap()
    w_d = nc.dram_tensor("w", [P, P], mybir.dt.float32, kind="ExternalInput").ap()
    y_d = nc.dram_tensor("y", [ROWS, HW], mybir.dt.float32, kind="ExternalOutput").ap()

    mm_dt = getattr(mybir.dt, mm_dtype_name)

    def mm_view(ap):
        return ap if mm_dtype_name == "float32" else ap.bitcast(mm_dt)

    with tile.TileContext(nc) as tc:
        with (
            tc.tile_pool(name="wp", bufs=1) as wp,
            tc.tile_pool(name="xp", bufs=3) as xp,
            tc.tile_pool(name="yp", bufs=3) as yp,
            tc.tile_pool(name="pp", bufs=8, space="PSUM") as pp,
        ):
            w_t = wp.tile([P, P], mybir.dt.float32)
            nc.sync.dma_start(out=w_t[:], in_=w_d[:])

            for g in range(N_GROUPS):
                r0 = g * P
                for ci in range(N_CHUNKS):
                    c0 = ci * CHUNK
                    xt = xp.tile([P, CHUNK], mybir.dt.float32, tag="xt")
                    nc.sync.dma_start(out=xt[:], in_=x_d[r0 : r0 + P, c0 : c0 + CHUNK])
                    yt = yp.tile([P, CHUNK], mybir.dt.float32, tag="yt")
                    for f0 in range(0, CHUNK, MM_N):
                        n = min(MM_N, CHUNK - f0)
                        ps = pp.tile([P, MM_N], mybir.dt.float32, tag="ps")
                        nc.tensor.matmul(
                            ps[:, :n],
                            mm_view(w_t[:]),
                            mm_view(xt[:, f0 : f0 + n]),
                            start=True,
                            stop=True,
                        )
                        nc.vector.tensor_copy(yt[:, f0 : f0 + n], ps[:, :n])
                    nc.scalar.dma_start(out=y_d[r0 : r0 + P, c0 : c0 + CHUNK], in_=yt[:])
    nc.compile()
    return nc


def _get_program():
    nc = _PROG_CACHE.get(MM_DTYPE)
    if nc is None:
        nc = _build_program(MM_DTYPE)
        _PROG_CACHE[MM_DTYPE] = nc
    return nc


def _weight_matrix(inhibition_filter, kronecker_delta):
    """126x126 block-diagonal lhsT = blockdiag(G.T, G.T), float32."""
    filt = np.asarray(inhibition_filter, dtype=np.float64).ravel()
    kd = np.asarray(kronecker_delta, dtype=np.float64).ravel()
    fk = np.fft.fft(kd - filt)
    g = np.real(np.fft.ifft(1.0 / fk))
    idx = (np.arange(C)[:, None] - np.arange(C)[None, :]) % C
    G = g[idx]  # G[c_out, c_in] = g[(c_out - c_in) mod C]
    lhsT = np.zeros((P, P), dtype=np.float32)
    GT = np.ascontiguousarray(G.T).astype(np.float32)  # lhsT[k, m] = G[m, k]
    lhsT[:C, :C] = GT
    lhsT[C:, C:] = GT
    return lhsT


LAST_RESULTS = None  # BassKernelResults of the most recent run (for profiling)


def kernel(activations, inhibition_filter, kronecker_delta):
    global LAST_RESULTS
    from concourse.bass_utils import run_bass_kernel_spmd

    acts = np.ascontiguousarray(np.asarray(activations, dtype=np.float32))
    assert acts.shape == (N_BATCH, C, H, W)
    w = _weight_matrix(inhibition_filter, kronecker_delta)

    nc = _get_program()
    in_maps = []
    for i in range(N_CORES):
        xs = acts[i * B_PER_CORE : (i + 1) * B_PER_CORE].reshape(ROWS, HW)
        in_maps.append({"x": np.ascontiguousarray(xs), "w": w})

    res = run_bass_kernel_spmd(nc, in_maps, list(range(N_CORES)))
    LAST_RESULTS = res

    out = np.concatenate(
        [res.results[i]["y"].reshape(B_PER_CORE, C, H, W) for i in range(N_CORES)],
        axis=0,
    )
    return out.astype(np.float32, copy=False)


# revision 10
# speedup vs baseline: 1.1303x; 1.1303x over previous
"""Converged-inhibition kernel for Trainium2 (8 NeuronCores, data-parallel).

The reference computes, per pixel (n, h, w):
    y = IFFT(FFT(x_c) / FFT(delta - filter)).real      (C = 63 channels)

Dividing by a fixed filter's DFT and inverse-transforming is a circular
deconvolution along the channel axis: y = G @ x with G the 63x63 circulant
matrix built from g = IFFT(1 / FFT(delta - filter)).real.  So the whole op
is one (63, 63) @ (63, N*H*W) matmul, embarrassingly parallel over pixels.

Device mapping: batch dim (64) sharded over 8 cores.  Since the contraction
dim (63) uses less than half the 128-wide PE array, two batches are stacked
per matmul column via a 126x126 block-diagonal weight, doubling PE
throughput.  The kernel is HBM-bandwidth bound (~50.6 MB per core).
"""

import os
import numpy as np

# Problem geometry (hardcoded: kernel.py must be self-contained).
C = 63
N_BATCH = 64
H = W = 112
HW = H * W                      # 12544
N_CORES = 8
B_PER_CORE = N_BATCH // N_CORES  # 8
P = 2 * C                       # 126 partitions = 2 batches stacked
ROWS = B_PER_CORE * C           # 504
N_GROUPS = B_PER_CORE // 2      # 4 batch-pairs per core
CHUNK = HW // 2                 # 6272 free-dim elements per pipelined chunk
N_CHUNKS = HW // CHUNK          # 2
MM_N = 512                      # moving free-dim per matmul (one PSUM bank)

# Matmul operand dtype: "float32" (safe) or "float32r" (fast fp32 PE path).
MM_DTYPE = os.environ.get("CI_MM_DTYPE", "float32")

_PROG_CACHE = {}


def _build_program(mm_dtype_name):
    import concourse.bacc as bacc
    import concourse.mybir as mybir
    from concourse import tile

    # Bacc (not raw Bass): its compile() splits multi-semaphore waits into
    # event-semaphore chains (HW allows only one wait per instruction).
    nc = bacc.Bacc("TRN2", target_bir_lowering=False, debug=False)
    # For float32r (fp32 with 11-bit mantissa, full-rate PE path) the BIR
    # verifier requires every matmul operand's producer to emit float32r —
    # declaring the DRAM inputs and SBUF tiles as float32r makes the DMA that
    # producer; the host pre-rounds the arrays to the representable set.
    mm_dt = getattr(mybir.dt, mm_dtype_name)
    x_d = nc.dram_tensor("x", [ROWS, HW], mm_dt, kind="ExternalInput").ap()
    w_d = nc.dram_tensor("w", [P, P], mm_dt, kind="ExternalInput").ap()
    y_d = nc.dram_tensor("y", [ROWS, HW], mybir.dt.float32, kind="ExternalOutput").ap()

    with tile.TileContext(nc) as tc:
        with (
            tc.tile_pool(name="wp", bufs=1) as wp,
            tc.tile_pool(name="xp", bufs=3) as xp,
            tc.tile_pool(name="yp", bufs=3) as yp,
            tc.tile_pool(name="pp", bufs=8, space="PSUM") as pp,
        ):
            w_t = wp.tile([P, P], mm_dt)
            nc.sync.dma_start(out=w_t[:], in_=w_d[:])

            for g in range(N_GROUPS):
                r0 = g * P
                for ci in range(N_CHUNKS):
                    c0 = ci * CHUNK
                    xt = xp.tile([P, CHUNK], mm_dt, tag="xt")
                    nc.sync.dma_start(out=xt[:], in_=x_d[r0 : r0 + P, c0 : c0 + CHUNK])
                    yt = yp.tile([P, CHUNK], mybir.dt.float32, tag="yt")
                    for f0 in range(0, CHUNK, MM_N):
                        n = min(MM_N, CHUNK - f0)
                        ps = pp.tile([P, MM_N], mybir.dt.float32, tag="ps")
                        nc.tensor.matmul(
                            ps[:, :n],
                            w_t[:],
                            xt[:, f0 : f0 + n],
                            start=True,
                            stop=True,
                        )
                        nc.vector.tensor_copy(yt[:, f0 : f0 + n], ps[:, :n])
                    nc.scalar.dma_start(out=y_d[r0 : r0 + P, c0 : c0 + CHUNK], in_=yt[:])
    nc.compile()
    return nc


def _get_program():
    nc = _PROG_CACHE.get(MM_DTYPE)
    if nc is None:
        nc = _build_program(MM_DTYPE)
        _PROG_CACHE[MM_DTYPE] = nc
    return nc


def _weight_matrix(inhibition_filter, kronecker_delta):
    """126x126 block-diagonal lhsT = blockdiag(G.T, G.T), float32."""
    filt = np.asarray(inhibition_filter, dtype=np.float64).ravel()
    kd = np.asarray(kronecker_delta, dtype=np.float64).ravel()
    fk = np.fft.fft(kd - filt)
    g = np.real(np.fft.ifft(1.0 / fk))
    idx = (np.arange(C)[:, None] - np.arange(C)[None, :]) % C
    G = g[idx]  # G[c_out, c_in] = g[(c_out - c_in) mod C]
    lhsT = np.zeros((P, P), dtype=np.float32)
    GT = np.ascontiguousarray(G.T).astype(np.float32)  # lhsT[k, m] = G[m, k]
    lhsT[:C, :C] = GT
    lhsT[C:, C:] = GT
    return lhsT


def _round_fp32r(a):
    """Round fp32 to float32r's representable set (11-bit mantissa, RNE)."""
    b = a.view(np.uint32)
    lsb = (b >> 12) & 1
    out = ((b + 0x7FF + lsb) & 0xFFFFF000).astype(np.uint32)
    return out.view(np.float32)


LAST_RESULTS = None  # BassKernelResults of the most recent run (for profiling)


def kernel(activations, inhibition_filter, kronecker_delta):
    global LAST_RESULTS
    from concourse.bass_utils import run_bass_kernel_spmd

    acts = np.ascontiguousarray(np.asarray(activations, dtype=np.float32))
    assert acts.shape == (N_BATCH, C, H, W)
    w = _weight_matrix(inhibition_filter, kronecker_delta)
    if MM_DTYPE == "float32r":
        acts = _round_fp32r(acts)
        w = _round_fp32r(w)

    nc = _get_program()
    in_maps = []
    for i in range(N_CORES):
        xs = acts[i * B_PER_CORE : (i + 1) * B_PER_CORE].reshape(ROWS, HW)
        in_maps.append({"x": np.ascontiguousarray(xs), "w": w})

    res = run_bass_kernel_spmd(nc, in_maps, list(range(N_CORES)))
    LAST_RESULTS = res

    out = np.concatenate(
        [res.results[i]["y"].reshape(B_PER_CORE, C, H, W) for i in range(N_CORES)],
        axis=0,
    )
    return out.astype(np.float32, copy=False)
